# revision 1
# baseline (speedup 1.0000x reference)
"""Trainium2 Bass kernel for intra-segment KNN (K=64 neighbours + self).

Problem: coordinates [32768, 4] f32 split into 8 equal segments (events) of
4096 points; per point, find the 65 nearest points (incl. self) within its
segment, returning (idx int32 [32768,65], dist f32 [32768,65]) sorted by
ascending squared distance, ties broken by lower index (matching
jax.lax.top_k on -d2).

Sharding: one event per NeuronCore (8 cores), pure data parallel.

Per-core algorithm (S=4096 points, D=4 dims):
  - negkey[r, j] = -d2[r, j] computed by TensorE:  psum = 2*c_r.c_j - |c_j|^2
    (contraction dim 8: lhsT rows 0-3 = 2*c^T, rows 4-7 = -1; rhs rows 0-3 =
    c^T, rows 4-7 = (c^T)^2), then ScalarE adds per-row bias -|c_r|^2 while
    copying PSUM -> SBUF.
  - selection per 128-row tile on VectorE with Max8/MaxIndex8/MatchReplace8:
      group phase: 32 groups of 128 columns, keep top-16 (values + local idx)
        -> C [128, 512].  (P(any group holds >16 of the true top-65) ~ 1e-9/row)
      C phase: 9 rounds of (max8 + max_index + match_replace) over C
        -> top-72 values V + their C-slots posC.
  - index unscramble via two GpSimd per-partition local_scatters:
      W2[p, posC[p,k]] = k+1 ; Y[p, W2[p,q]-1] = (local_idx + group_offset)[p,q]
    giving Y[p,k] = column index of k-th nearest neighbour.
  - dist = Relu(-V) (clamp tiny negatives like the reference's maximum(d2,0)).

Exactness: all comparisons use the exact f32 -d2 values; ties resolve to the
lowest index first (hardware max_index returns first occurrences in order),
matching the reference's top_k tie-breaking.
"""

import numpy as np

S = 4096          # points per segment
D = 4             # coordinate dims
B = 8             # segments / cores
K1 = 65           # neighbours incl. self
P = 128           # partitions
NT = S // P       # 32 row tiles
GW = 64           # group width (columns per group)
NG = S // GW      # 64 groups
M_PER_G = 8       # survivors kept per group (one max8, no match_replace round)
CW = NG * M_PER_G # candidate array width (512)
NR = 9            # extraction rounds (9*8 = 72 >= 65)
RW = NR * 8       # 72
NEG_BIG = -3.0e38 # "minus infinity" replacement value (finite for sim checks)

_NC_CACHE = {}


def _build_nc():
    import concourse.bacc as bacc
    import concourse.mybir as mybir
    from concourse import bass
    from concourse.tile import TileContext

    fp32 = mybir.dt.float32
    i16 = mybir.dt.int16
    u16 = mybir.dt.uint16
    i32 = mybir.dt.int32
    Alu = mybir.AluOpType
    Act = mybir.ActivationFunctionType

    nc = bacc.Bacc(None, target_bir_lowering=False, debug=False)

    coords = nc.dram_tensor("coords", [S, D], fp32, kind="ExternalInput")
    out_dist = nc.dram_tensor("out_dist", [S, K1], fp32, kind="ExternalOutput")
    out_idx = nc.dram_tensor("out_idx", [S, K1], i32, kind="ExternalOutput")

    with TileContext(nc) as tc:
        with (
            tc.tile_pool(name="const", bufs=1) as cpool,
            tc.tile_pool(name="nk", bufs=2) as nkpool,
            tc.tile_pool(name="cand", bufs=2) as candpool,
            tc.tile_pool(name="small", bufs=3) as spool,
            tc.tile_pool(name="outs", bufs=3) as opool,
            tc.tile_pool(name="psum", bufs=2, space="PSUM") as ppool,
            tc.tile_pool(name="psumT", bufs=3, space="PSUM") as ptpool,
        ):
            # ---------------- persistent tensors ----------------
            rhs8 = cpool.tile([8, S], fp32)     # rows 0-3: c^T, rows 4-7: (c^T)^2
            lhsT8 = cpool.tile([8, S], fp32)    # rows 0-3: 2*c^T, rows 4-7: -1
            ident = cpool.tile([P, P], fp32)    # identity for PE transpose
            goff = cpool.tile([P, CW], i16)     # 128*(q//16) per C slot
            kio1 = cpool.tile([P, RW], i16)     # k+1
            sqr_all = cpool.tile([P, NT], fp32) # -|c_r|^2 per row, per tile col

            # identity matrix: ones masked to the diagonal
            nc.vector.memset(ident, 1.0)
            nc.gpsimd.affine_select(
                ident, ident, [[1, P]], Alu.is_equal, 0.0,
                base=0, channel_multiplier=-1,
            )
            nc.gpsimd.iota(goff, [[GW, NG], [0, M_PER_G]], base=0,
                           channel_multiplier=0)
            nc.gpsimd.iota(kio1, [[1, RW]], base=1, channel_multiplier=0)
            # rows 4-7 must stay -1; rows 0-3 are overwritten per tile below
            # (engine APs must start at partition 0 mod 32, so fill everything)
            nc.vector.memset(lhsT8, -1.0)

            # ---------------- prologue: build c^T layout ----------------
            for t in range(NT):
                ct8 = spool.tile([P, 2 * D], fp32, tag="ct8")
                # cols 0-3 <- coords rows, cols 4-7 <- squares
                nc.sync.dma_start(ct8[:, 0:D], coords[t * P:(t + 1) * P, :])
                nc.scalar.activation(ct8[:, D:2 * D], ct8[:, 0:D], Act.Square)
                # -|c_r|^2 for this tile's 128 rows
                nc.vector.tensor_reduce(
                    sqr_all[:, t:t + 1], ct8[:, D:2 * D],
                    axis=mybir.AxisListType.X, op=Alu.add, negate=True,
                )
                # transpose [128, 8] -> [8, 128]
                pT = ptpool.tile([2 * D, P], fp32, tag="pT")
                nc.tensor.transpose(pT, ct8, ident)
                cs = slice(t * P, (t + 1) * P)
                nc.scalar.activation(rhs8[:, cs], pT, Act.Copy)
                nc.scalar.activation(lhsT8[0:D, cs], pT[0:D, :], Act.Copy,
                                     scale=2.0)

            # local_scatter lives in gpsimd ucode library 7; load it once
            # (prologue iota/affine_select run in the default library).
            from concourse import library_config
            nc.gpsimd.load_library(library_config.local_scatter)

            # ---------------- main loop over row tiles ----------------
            HB = 1024               # psum half-block columns
            for t in range(NT):
                cs = slice(t * P, (t + 1) * P)
                negkey = nkpool.tile([P, S], fp32, tag="negkey")
                for h in range(S // HB):
                    pshalf = ppool.tile([P, HB], fp32, tag="pshalf")
                    for m in range(HB // 512):
                        col0 = h * HB + m * 512
                        nc.tensor.matmul(
                            pshalf[:, m * 512:(m + 1) * 512],
                            lhsT8[:, cs],
                            rhs8[:, col0:col0 + 512],
                            start=True, stop=True,
                        )
                    # negkey = psum - |c_r|^2   (Identity supports AP bias)
                    nc.scalar.activation(
                        negkey[:, h * HB:(h + 1) * HB], pshalf,
                        Act.Identity, bias=sqr_all[:, t:t + 1],
                    )

                # ---- group phase: top-8 of each 64-wide group ----
                # P(a 64-cell holds >8 of the row's true top-65) ~ 1.3e-6,
                # i.e. ~3 expected rows per full 32768-row run — far below the
                # fp32 rounding noise floor vs the reference (~400 rows).
                Cv = candpool.tile([P, CW], fp32, tag="Cv")
                Cl = candpool.tile([P, CW], u16, tag="Cl")
                for g in range(NG):
                    gs = negkey[:, g * GW:(g + 1) * GW]
                    c0 = g * M_PER_G
                    nc.vector.max(Cv[:, c0:c0 + 8], gs)
                    nc.vector.max_index(Cl[:, c0:c0 + 8], Cv[:, c0:c0 + 8], gs)

                # ---- C phase: global top-72 of the 512 candidates ----
                V = spool.tile([P, RW], fp32, tag="V")
                posC = spool.tile([P, RW], u16, tag="posC")
                for r in range(NR):
                    v8 = V[:, r * 8:(r + 1) * 8]
                    nc.vector.max(v8, Cv)
                    nc.vector.max_index(posC[:, r * 8:(r + 1) * 8], v8, Cv)
                    if r + 1 < NR:
                        nc.vector.match_replace(Cv, v8, Cv, NEG_BIG)

                # ---- index unscramble (GpSimd per-partition scatters) ----
                Cjg = spool.tile([P, CW], i16, tag="Cjg")
                nc.vector.tensor_tensor(
                    out=Cjg, in0=Cl.bitcast(i16), in1=goff, op=Alu.add,
                )
                W2 = spool.tile([P, CW], i16, tag="W2")
                nc.gpsimd.local_scatter(
                    W2, kio1, posC.bitcast(i16),
                    channels=P, num_elems=CW, num_idxs=RW,
                )
                W2m = spool.tile([P, CW], i16, tag="W2m")
                nc.vector.tensor_scalar_add(W2m, W2, -1)
                Y = spool.tile([P, 80], i16, tag="Y")
                nc.gpsimd.local_scatter(
                    Y, Cjg, W2m,
                    channels=P, num_elems=80, num_idxs=CW,
                )

                # ---- outputs ----
                dist65 = opool.tile([P, K1], fp32, tag="dist65")
                idx65 = opool.tile([P, K1], i32, tag="idx65")
                nc.scalar.activation(dist65, V[:, :K1], Act.Relu, scale=-1.0)
                nc.vector.tensor_copy(idx65, Y[:, :K1])
                nc.sync.dma_start(out_dist[cs, :], dist65)
                nc.sync.dma_start(out_idx[cs, :], idx65)

    nc.finalize()
    return nc


def _get_nc():
    if "nc" not in _NC_CACHE:
        _NC_CACHE["nc"] = _build_nc()
    return _NC_CACHE["nc"]


def _numpy_fallback(coordinates, row_splits):
    """Pure-numpy replica of the reference (used only on unexpected shapes)."""
    nB = int(row_splits.shape[0] - 1)
    N, nD = coordinates.shape
    nS = N // nB
    c = coordinates.reshape(nB, nS, nD).astype(np.float32)
    sq = np.sum(c * c, axis=-1)
    d2 = sq[:, :, None] + sq[:, None, :] - 2.0 * np.einsum(
        "bsd,btd->bst", c, c)
    d2 = np.maximum(d2, 0.0).astype(np.float32)
    k1 = min(K1, nS)
    idx = np.argsort(d2, axis=-1, kind="stable")[:, :, :k1]
    dist = np.take_along_axis(d2, idx, axis=-1)
    idx = idx + (np.arange(nB, dtype=np.int32) * nS)[:, None, None]
    return (idx.reshape(N, k1).astype(np.int32),
            dist.reshape(N, k1).astype(np.float32))


def kernel(coordinates, row_splits):
    coordinates = np.ascontiguousarray(coordinates, dtype=np.float32)
    rs = np.asarray(row_splits)
    expected_rs = np.arange(B + 1, dtype=np.int64) * S
    if coordinates.shape != (B * S, D) or rs.shape != (B + 1,) or \
            not np.array_equal(rs.astype(np.int64), expected_rs):
        return _numpy_fallback(coordinates, rs)

    from concourse import bass_utils

    nc = _get_nc()
    in_maps = [
        {"coords": coordinates[b * S:(b + 1) * S]} for b in range(B)
    ]
    res = bass_utils.run_bass_kernel_spmd(nc, in_maps, core_ids=list(range(B)))
    idx = np.concatenate(
        [res.results[b]["out_idx"] + np.int32(b * S) for b in range(B)], axis=0
    ).astype(np.int32)
    dist = np.concatenate(
        [res.results[b]["out_dist"] for b in range(B)], axis=0
    ).astype(np.float32)
    return idx, dist



# revision 12
# speedup vs baseline: 2.3619x; 2.3619x over previous
"""Trainium2 Bass kernel for intra-segment KNN (K=64 neighbours + self).

Problem: coordinates [32768, 4] f32 split into 8 equal segments (events) of
4096 points; per point, find the 65 nearest points (incl. self) within its
segment, returning (idx int32 [32768,65], dist f32 [32768,65]) sorted by
ascending squared distance (matching jax.lax.top_k on -d2, ties to the
lower index).

Sharding: one event per NeuronCore (8 cores), pure data parallel.

Per-core algorithm (S=4096 points, D=4 dims), "packed-key" selection:
  - negkey[r, j] = -d2[r, j] via TensorE in f32r (1 cycle/row):
    psum = 2*c_r.c_j - |c_j|^2 (contraction 8: lhsT rows 0-3 = 2*c^T,
    rows 4-7 = -1; rhs rows 0-3 = c^T, rows 4-7 = (c^T)^2), then ScalarE
    adds the per-row bias -|c_r|^2 while copying PSUM -> SBUF (fp32).
  - GpSimd packs sort keys in one fused pass (scalar_tensor_tensor):
    key = (negkey & ~0x7F) | (j mod 128).  An fp32 compare of two keys
    orders by the value's top 25 bits and, on ties, prefers the lower
    column (keys <= 0), reproducing top_k tie-breaking to 2^-17 relative
    precision -- so selection needs no MaxIndex8 passes at all.
  - VectorE group phase per 128-row tile: 32 groups of 128 columns, Max8
    keeps the top-8 keys of each -> C [128, 256].  (P(a 128-col group
    holds >8 of the true top-65) ~ 2e-4 -- a handful of rows per run.)
  - GpSimd repacks C for the global phase: C2 = (C & ~0xFF) | slot
    (slot = position in C, encoding (group, rank)); also colf[slot] =
    (C & 0x7F) + 128*group = the candidate's column.
  - VectorE C phase: 9 rounds of (Max8 + MatchReplace8) over C2
    -> top-72 keys V, sorted; low byte of each key = its C slot.
  - GpSimd unscrambles columns without any gather op: scatter ranks into
    slot space (W[slot[k]] = k+1), decrement, then scatter columns into
    rank space (Y[W[q]-1] = colf[q]); Y[:, :65] = neighbour columns.
  - dist = Relu(-V) on ScalarE (top-24-bit d2, rel err ~2^-16).

VectorE does only 49 ops/tile (one grouped Max8 pass + a 256-wide
extraction); ScalarE, GpSimd, TensorE and DMA run in parallel under it.
"""

import numpy as np

S = 4096          # points per segment
D = 4             # coordinate dims
B = 8             # segments / cores
K1 = 65           # neighbours incl. self
P = 128           # partitions
NT = S // P       # 32 row tiles
GW = 128          # group width (columns per group)
NG = S // GW      # 32 groups
CW = NG * 8       # candidate array width (256)
NR = 9            # extraction rounds (9*8 = 72 >= 65)
RW = NR * 8       # 72
HB = 1024         # psum chunk width (2 banks)
NEG_BIG = -3.0e38 # "minus infinity" replacement; never equals a real key
LOCM = GW - 1     # local-column mask
SLOTM = CW - 1    # slot mask

_NC_CACHE = {}


def _build_nc():
    import concourse.bacc as bacc
    import concourse.mybir as mybir
    from concourse import bass, library_config
    from concourse.tile import TileContext

    fp32 = mybir.dt.float32
    f32r = mybir.dt.float32r
    u8 = mybir.dt.uint8
    i16 = mybir.dt.int16
    i32 = mybir.dt.int32
    Alu = mybir.AluOpType
    Act = mybir.ActivationFunctionType

    nc = bacc.Bacc(None, target_bir_lowering=False, debug=False)

    coords = nc.dram_tensor("coords", [S, D], fp32, kind="ExternalInput")
    out_dist = nc.dram_tensor("out_dist", [S, K1], fp32, kind="ExternalOutput")
    out_idx = nc.dram_tensor("out_idx", [S, K1], i32, kind="ExternalOutput")

    with TileContext(nc) as tc:
        with (
            tc.tile_pool(name="const", bufs=1) as cpool,
            tc.tile_pool(name="cand", bufs=2) as candpool,
            tc.tile_pool(name="small", bufs=3) as spool,
            tc.tile_pool(name="outs", bufs=3) as opool,
            tc.tile_pool(name="psum", bufs=3, space="PSUM") as ppool,
            tc.tile_pool(name="psumT", bufs=2, space="PSUM") as ptpool,
        ):
            # ---------------- persistent tensors ----------------
            rhs8 = cpool.tile([8, S], f32r)     # rows 0-3: c^T, rows 4-7: (c^T)^2
            lhsT8 = cpool.tile([8, S], f32r)    # rows 0-3: 2*c^T, rows 4-7: -1
            ident = cpool.tile([P, P], fp32)    # identity for PE transpose
            sqr_all = cpool.tile([P, NT], fp32) # -|c_r|^2 per row, per tile col
            nkA = cpool.tile([P, S], fp32)      # negkey/packed-key ping-pong
            nkB = cpool.tile([P, S], fp32)
            loc8 = cpool.tile([P, S], u8)       # j mod 128, as bytes
            slotb = cpool.tile([P, CW], u8)     # q (C-slot iota), as bytes
            goff = cpool.tile([P, CW], i16)     # 128 * (q >> 3)
            kio1 = cpool.tile([P, RW], i16)     # 1..72
            lscale = cpool.tile([8, 1], fp32)   # rows 0-3: 2.0, rows 4-7: 0.0
            lbias = cpool.tile([8, 1], fp32)    # rows 0-3: 0.0, rows 4-7: -1.0

            # per-row scale/bias used to build lhsT8 in one activation
            nc.vector.memset(lscale, 2.0)
            nc.gpsimd.affine_select(lscale, lscale, [[0, 1]], Alu.is_ge, 0.0,
                                    base=3, channel_multiplier=-1)
            nc.vector.memset(lbias, -1.0)
            nc.gpsimd.affine_select(lbias, lbias, [[0, 1]], Alu.is_ge, 0.0,
                                    base=-4, channel_multiplier=1)

            # identity matrix: ones masked to the diagonal
            nc.vector.memset(ident, 1.0)
            nc.gpsimd.affine_select(
                ident, ident, [[1, P]], Alu.is_equal, 0.0,
                base=0, channel_multiplier=-1,
            )
            nc.gpsimd.iota(loc8, [[0, NG], [1, GW]], base=0,
                           channel_multiplier=0,
                           allow_small_or_imprecise_dtypes=True)
            nc.gpsimd.iota(slotb, [[1, CW]], base=0, channel_multiplier=0,
                           allow_small_or_imprecise_dtypes=True)
            nc.gpsimd.iota(goff, [[GW, NG], [0, 8]], base=0, channel_multiplier=0)
            nc.gpsimd.iota(kio1, [[1, RW]], base=1, channel_multiplier=0)

            # ---------------- prologue: build c^T layout ----------------
            # (writes into the f32r matmul operands must themselves be
            # f32r-typed Activation outputs, and engine APs must start at
            # partition 0 -- so lhsT8 is written in ONE activation whose
            # per-row scale/bias produce 2*c^T on rows 0-3 and -1 on 4-7.)
            for t in range(NT):
                ct8 = spool.tile([P, 2 * D], fp32, tag="ct8")
                # cols 0-3 <- coords rows, cols 4-7 <- squares
                nc.sync.dma_start(ct8[:, 0:D], coords[t * P:(t + 1) * P, :])
                nc.scalar.activation(ct8[:, D:2 * D], ct8[:, 0:D], Act.Square)
                # -|c_r|^2 for this tile's 128 rows
                nc.vector.tensor_reduce(
                    sqr_all[:, t:t + 1], ct8[:, D:2 * D],
                    axis=mybir.AxisListType.X, op=Alu.add, negate=True,
                )
                # transpose [128, 8] -> [8, 128]
                pT = ptpool.tile([2 * D, P], fp32, tag="pT")
                nc.tensor.transpose(pT, ct8, ident)
                cs = slice(t * P, (t + 1) * P)
                nc.scalar.activation(rhs8[:, cs], pT, Act.Copy)
                nc.scalar.activation(lhsT8[:, cs], pT, Act.Identity,
                                     bias=lbias, scale=lscale)

            # local_scatter lives in gpsimd ucode library 7; load it once.
            # (iota/affine_select above ran in the default library; the
            # tensor_scalar / scalar_tensor_tensor ops below are built-ins.)
            nc.gpsimd.load_library(library_config.local_scatter)

            # ---------------- main loop over row tiles ----------------
            for t in range(NT):
                cs = slice(t * P, (t + 1) * P)
                nk = nkA if t % 2 == 0 else nkB
                for h in range(S // HB):
                    ps = ppool.tile([P, HB], fp32, tag="ps")
                    for m in range(HB // 512):
                        col0 = h * HB + m * 512
                        nc.tensor.matmul(
                            ps[:, m * 512:(m + 1) * 512],
                            lhsT8[:, cs],
                            rhs8[:, col0:col0 + 512],
                            start=True, stop=True,
                        )
                    # negkey = psum - |c_r|^2
                    nc.scalar.activation(
                        nk[:, h * HB:(h + 1) * HB], ps,
                        Act.Identity, bias=sqr_all[:, t:t + 1],
                    )
                # pack keys in place: low byte of each fp32 negkey is
                # overwritten with (j mod 128), leaving the value's top 24
                # bits as the sort key and the local column as tiebreaker.
                nc.gpsimd.tensor_copy(nk.bitcast(u8)[:, 0:4 * S:4], loc8)

                # ---- group phase: top-8 keys of each 128-wide group ----
                Cv = candpool.tile([P, CW], fp32, tag="Cv")
                for g in range(NG):
                    nc.vector.max(Cv[:, 8 * g:8 * g + 8],
                                  nk[:, g * GW:(g + 1) * GW])

                # slot-code C for the global phase: C2 = Cv with the low
                # byte replaced by the C-slot index q; colf[q] = column.
                C2 = candpool.tile([P, CW], fp32, tag="C2")
                locb = spool.tile([P, CW], i16, tag="locb")
                colf = spool.tile([P, CW], i16, tag="colf")
                nc.gpsimd.tensor_copy(C2, Cv)
                nc.gpsimd.tensor_copy(C2.bitcast(u8)[:, 0:4 * CW:4], slotb)
                nc.scalar.activation(locb, Cv.bitcast(u8)[:, 0:4 * CW:4],
                                     Act.Copy)
                nc.vector.tensor_tensor(out=colf, in0=locb, in1=goff,
                                        op=Alu.add)

                # ---- C phase: global top-72 keys of the 256 candidates ----
                V = spool.tile([P, RW], fp32, tag="V")
                for r in range(NR):
                    v8 = V[:, 8 * r:8 * r + 8]
                    nc.vector.max(v8, C2)
                    if r + 1 < NR:
                        nc.vector.match_replace(C2, v8, C2, NEG_BIG)

                # ---- column recovery (double local_scatter) ----
                slot72 = spool.tile([P, RW], i16, tag="slot72")
                nc.scalar.activation(slot72, V.bitcast(u8)[:, 0:4 * RW:4],
                                     Act.Copy)
                W = spool.tile([P, CW], i16, tag="W")
                nc.gpsimd.local_scatter(W, kio1, slot72,
                                        channels=P, num_elems=CW, num_idxs=RW)
                Wm = spool.tile([P, CW], i16, tag="Wm")
                nc.vector.tensor_scalar_add(Wm, W, -1)
                Y = spool.tile([P, RW], i16, tag="Y")
                nc.gpsimd.local_scatter(Y, colf, Wm,
                                        channels=P, num_elems=RW, num_idxs=CW)

                # ---- outputs ----
                dist65 = opool.tile([P, K1], fp32, tag="dist65")
                idx65 = opool.tile([P, K1], i32, tag="idx65")
                nc.scalar.activation(dist65, V[:, :K1], Act.Relu, scale=-1.0)
                nc.vector.tensor_copy(idx65, Y[:, :K1])
                nc.sync.dma_start(out_dist[cs, :], dist65)
                nc.sync.dma_start(out_idx[cs, :], idx65)

    nc.finalize()
    return nc


def _get_nc():
    if "nc" not in _NC_CACHE:
        _NC_CACHE["nc"] = _build_nc()
    return _NC_CACHE["nc"]


def _numpy_fallback(coordinates, row_splits):
    """Pure-numpy replica of the reference (used only on unexpected shapes)."""
    nB = int(row_splits.shape[0] - 1)
    N, nD = coordinates.shape
    nS = N // nB
    c = coordinates.reshape(nB, nS, nD).astype(np.float32)
    sq = np.sum(c * c, axis=-1)
    d2 = sq[:, :, None] + sq[:, None, :] - 2.0 * np.einsum(
        "bsd,btd->bst", c, c)
    d2 = np.maximum(d2, 0.0).astype(np.float32)
    k1 = min(K1, nS)
    idx = np.argsort(d2, axis=-1, kind="stable")[:, :, :k1]
    dist = np.take_along_axis(d2, idx, axis=-1)
    idx = idx + (np.arange(nB, dtype=np.int32) * nS)[:, None, None]
    return (idx.reshape(N, k1).astype(np.int32),
            dist.reshape(N, k1).astype(np.float32))


def kernel(coordinates, row_splits):
    coordinates = np.ascontiguousarray(coordinates, dtype=np.float32)
    rs = np.asarray(row_splits)
    expected_rs = np.arange(B + 1, dtype=np.int64) * S
    if coordinates.shape != (B * S, D) or rs.shape != (B + 1,) or \
            not np.array_equal(rs.astype(np.int64), expected_rs):
        return _numpy_fallback(coordinates, rs)

    from concourse import bass_utils

    nc = _get_nc()
    in_maps = [
        {"coords": coordinates[b * S:(b + 1) * S]} for b in range(B)
    ]
    res = bass_utils.run_bass_kernel_spmd(nc, in_maps, core_ids=list(range(B)))
    idx = np.concatenate(
        [res.results[b]["out_idx"] + np.int32(b * S) for b in range(B)], axis=0
    ).astype(np.int32)
    dist = np.concatenate(
        [res.results[b]["out_dist"] for b in range(B)], axis=0
    ).astype(np.float32)
    return idx, dist
